# revision 39
# baseline (speedup 1.0000x reference)
"""RGCN graph-scoring kernel for Trainium2 (8 NeuronCores, one graph per core).

Math (per graph):
  out = relu(x @ root + bias + sum_r mean_r @ W_r);  scores = out @ lin + linb
  mean_r[n] = mean of x[src_e] over edges e with dst_e == n, type_e == r.

Device strategy per core (v2 — bf16 + 4 SWDGE queues + host-built one-hots):
  1. Phase 1: xw[src*8 + r_local] = (x @ W_r)[src] in bf16, staged to DRAM
     (two halves r<8 / r>=8 so gather indices fit in int16). PSUM->SBUF
     copies alternate between the Scalar and Vector engines.
  2. Phase 2, per dst tile t: dma_gather edge rows z_e = xw[src_e, type_e]
     (gathers round-robin across 4 SWDGE queues so descriptor generation
     runs on multiple Q7 core pairs concurrently), then accumulate
       acc[c', m] += z_chunk^T @ OH_chunk
     where OH_chunk[e, m] = alpha_e * (dstloc_e == m) is PRECOMPUTED ON THE
     HOST in bf16 and DMA-streamed (alpha_e = 1/cnt(type_e, dst_e) folds the
     mean normalization; padding slots have alpha 0 and index 0).
     acc is seeded by the root-weight matmul; relu+bias on ACT; scores via
     an M=1 matmul with the linear head.

Host side shards graphs across cores, sorts edges by (dst_tile, r_half) into
per-bin chunk counts shared across all 8 cores (max over cores, rounded up to
128) so the SPMD program is identical, and packs index/one-hot arrays.
"""

import sys

for _p in ("/opt/trn_rl_repo", "/root/.axon_site/_ro/trn_rl_repo"):
    if _p not in sys.path:
        sys.path.insert(0, _p)

import numpy as np
import ml_dtypes

import concourse.bacc as bacc
import concourse.mybir as mybir
from concourse.tile import TileContext
from concourse.bass_utils import run_bass_kernel_spmd

P = 128
B, N, C, R, E = 8, 4096, 128, 16, 65536
NT = N // P  # 32 dst node tiles
NH = 2  # r halves (int16 gather index limit: src*8+rl < 32768)
RH = R // NH  # 8 relations per half
NBINS = NT * NH
NQ = 4  # SWDGE queues
Z1B = 12  # z1 pool depth (h=1 gather buffers)

BF16 = ml_dtypes.bfloat16

_prog_cache = {}


def build_program(nch):
    """Build + compile the SPMD Bass program for per-bin chunk counts `nch`
    (tuple of NBINS ints, shared by all cores)."""
    nch = tuple(int(v) for v in nch)
    G = sum(nch)  # total 128-edge chunks
    off = np.zeros(NBINS + 1, np.int64)
    off[1:] = np.cumsum(nch)

    nc = bacc.Bacc("TRN2", num_swdge_queues=NQ)
    f32 = mybir.dt.float32
    bf16 = mybir.dt.bfloat16

    xT = nc.dram_tensor("xT", [P, N], bf16, kind="ExternalInput")
    wcat = nc.dram_tensor("wcat", [P, R * C], bf16, kind="ExternalInput")
    root = nc.dram_tensor("root", [P, C], bf16, kind="ExternalInput")
    bias = nc.dram_tensor("bias", [P, 1], f32, kind="ExternalInput")
    lin = nc.dram_tensor("lin", [P, 1], bf16, kind="ExternalInput")
    linb = nc.dram_tensor("linb", [1, 1], f32, kind="ExternalInput")
    gidx = nc.dram_tensor("gidx", [P, G * 8], mybir.dt.int16, kind="ExternalInput")
    oh = nc.dram_tensor("oh", [P, G * P], bf16, kind="ExternalInput")
    scores = nc.dram_tensor("scores", [1, N], f32, kind="ExternalOutput")

    with TileContext(nc) as tc:
        with (
            tc.tile_pool(name="const", bufs=1) as cpool,
            tc.tile_pool(name="stage", bufs=3) as spool,
            tc.tile_pool(name="z0", bufs=NT) as zpool0,
            tc.tile_pool(name="z1", bufs=Z1B) as zpool1,
            tc.tile_pool(name="oh", bufs=6) as ohpool,
            tc.tile_pool(name="post", bufs=4) as ppool,
            tc.tile_pool(name="pxw", bufs=3, space="PSUM") as pxw_pool,
            tc.tile_pool(name="pacc", bufs=4, space="PSUM") as pacc_pool,
            tc.tile_pool(name="plin", bufs=1, space="PSUM") as plin_pool,
            tc.tile_pool(name="dram", bufs=1, space="DRAM") as dpool,
        ):
            # ---- resident inputs ----
            # xT/wcat gate the first phase-1 matmul: split them across the two
            # HWDGE rings so they land as early as possible.
            xT_t = cpool.tile([P, N], bf16)
            nc.sync.dma_start(out=xT_t[:, : N // 2], in_=xT[:, : N // 2])
            nc.scalar.dma_start(out=xT_t[:, N // 2 :], in_=xT[:, N // 2 :])
            wcat_t = cpool.tile([P, R * C], bf16)
            nc.sync.dma_start(out=wcat_t[:], in_=wcat[:])
            root_t = cpool.tile([P, C], bf16)
            nc.sync.dma_start(out=root_t[:], in_=root[:])
            bias_t = cpool.tile([P, 1], f32)
            nc.sync.dma_start(out=bias_t[:], in_=bias[:])
            lin_t = cpool.tile([P, 1], bf16)
            nc.sync.dma_start(out=lin_t[:], in_=lin[:])
            linb_t = cpool.tile([1, 1], f32)
            nc.sync.dma_start(out=linb_t[:], in_=linb[:])
            # idx feeds the gathers (not needed before ~40us) — keep it off the
            # Sync ring's critical phase-1a stretch.
            idx_t = cpool.tile([P, G * 8], mybir.dt.int16)
            nc.scalar.dma_start(out=idx_t[:], in_=gidx[:])

            # DRAM scratch: per-half transformed features, row = src*8 + r_local
            xw = [
                dpool.tile([N * RH, C], bf16, name=f"xw{h}", tag=f"xw{h}")
                for h in range(NH)
            ]

            # Pre-issue the first OH loads on the ACT HWDGE ring (Sync's ring
            # carries the inputs + xw[0] writes) — they have no dependencies
            # and phase 2's first accumulations need them early.
            oh_tiles = [None] * NT
            OH_PRE = 6
            for t in range(OH_PRE):
                b0 = t * NH
                tch = nch[b0] + nch[b0 + 1]
                oh_tiles[t] = ohpool.tile([P, tch * P], bf16, name=f"oh{t}", tag="oh")
                nc.scalar.dma_start(
                    out=oh_tiles[t][:],
                    in_=oh[:, off[b0] * P : (off[b0] + tch) * P],
                )

            # ---- phase 1: xw = x @ W_r (bf16), staged out to DRAM ----
            # h-major so all of xw[0] lands first and the h=0 gathers can
            # start while the h=1 half is still being computed. xw writes are
            # batched 4 src tiles per DMA to keep the Sync engine's issue rate
            # off the critical path.
            SG = 4  # src tiles per staged write
            for h in range(NH):
                for sg in range(NT // SG):
                    stg = spool.tile([P, SG, RH * C], bf16, tag="stage")
                    for si in range(SG):
                        st = sg * SG + si
                        for g in range(2):
                            pxw = pxw_pool.tile([P, 512], f32, space="PSUM", tag="pxw")
                            nc.tensor.matmul(
                                out=pxw[:],
                                lhsT=xT_t[:, st * P : (st + 1) * P],
                                rhs=wcat_t[:, h * RH * C + g * 512 : h * RH * C + (g + 1) * 512],
                                start=True,
                                stop=True,
                            )
                            # alternate ACT/DVE on the PSUM->SBUF cast so
                            # neither engine gates the PE
                            if g == 0:
                                nc.scalar.activation(
                                    out=stg[:, si, :512],
                                    in_=pxw[:],
                                    func=mybir.ActivationFunctionType.Copy,
                                )
                            else:
                                nc.vector.tensor_scalar(
                                    out=stg[:, si, 512:],
                                    in0=pxw[:],
                                    scalar1=0.0,
                                    scalar2=None,
                                    op0=mybir.AluOpType.add,
                                )
                    # xw row index is (src%128)*256 + (src//128)*8 + rl so each
                    # staged write is one contiguous 8KB run per partition
                    # (full-bandwidth DMA shape); the host computes gather
                    # indices with the same mapping.
                    dst_view = xw[h][:].rearrange(
                        "(p ng s rl) c -> ng p (s rl c)", p=P, ng=NT // SG, s=SG, rl=RH
                    )[sg]
                    # split write traffic across the two HWDGE rings: xw[0] on
                    # Sync (with the inputs), xw[1] on ACT (after OH preloads)
                    eng = nc.sync if h == 0 else nc.scalar
                    eng.dma_start(
                        out=dst_view,
                        in_=stg[:].rearrange("p s (rl c) -> p (s rl c)", rl=RH),
                    )

            # ---- phase 2: gather + aggregate per dst tile ----
            # All h=0 gathers are issued first: they only depend on xw[0], so
            # Q7 descriptor generation overlaps with phase 1's h=1 half.
            # Greedy static queue balancing: each gather goes to the Q7 pair
            # with the least accumulated index work so no pair straggles.
            qload = [0] * NQ
            qpick = []
            for b in list(range(0, NBINS, 2)) + list(range(1, NBINS, 2)):
                q = min(range(NQ), key=lambda i: (qload[i], i))
                qload[q] += nch[b]
                qpick.append(q)
            qit = iter(qpick)
            z_h0 = []
            for t in range(NT):
                b = t * NH
                cap = nch[b] * P
                z = zpool0.tile([P, nch[b], C], bf16, name=f"z0_{t}", tag="z0")
                nc.gpsimd.dma_gather(
                    z[:],
                    xw[0][:],
                    idx_t[:, off[b] * 8 : off[b] * 8 + cap // 16],
                    cap,
                    cap,
                    C,
                    single_packet=False,
                    queue_num=next(qit),
                )
                z_h0.append(z)

            scores_t = cpool.tile([1, N], f32)
            for t in range(NT):
                b0, b1 = t * NH, t * NH + 1
                tch = nch[b0] + nch[b1]  # chunks for this tile
                if oh_tiles[t] is None:
                    oh_tiles[t] = ohpool.tile([P, tch * P], bf16, name=f"oh{t}", tag="oh")
                    nc.sync.dma_start(
                        out=oh_tiles[t][:],
                        in_=oh[:, off[b0] * P : (off[b0] + tch) * P],
                    )
                oh_t = oh_tiles[t]
                cap1 = nch[b1] * P
                z1 = zpool1.tile([P, nch[b1], C], bf16, name=f"z1_{t}", tag="z1")
                nc.gpsimd.dma_gather(
                    z1[:],
                    xw[1][:],
                    idx_t[:, off[b1] * 8 : off[b1] * 8 + cap1 // 16],
                    cap1,
                    cap1,
                    C,
                    single_packet=False,
                    queue_num=next(qit),
                )
                acc = pacc_pool.tile([P, P], f32, space="PSUM", tag="acc")
                # root term seeds the accumulator (start=True clears the bank)
                nc.tensor.matmul(
                    out=acc[:],
                    lhsT=root_t[:],
                    rhs=xT_t[:, t * P : (t + 1) * P],
                    start=True,
                    stop=False,
                )
                for h, zt in ((0, z_h0[t]), (1, z1)):
                    b = t * NH + h
                    for j in range(nch[b]):
                        g_loc = (off[b] - off[b0]) + j
                        nc.tensor.matmul(
                            out=acc[:],
                            lhsT=zt[:, j, :],
                            rhs=oh_t[:, g_loc * P : (g_loc + 1) * P],
                            start=False,
                            stop=(h == NH - 1 and j == nch[b] - 1),
                        )
                # relu(acc + bias) -> SBUF bf16
                relu_t = ppool.tile([P, P], bf16, tag="relu")
                nc.scalar.activation(
                    out=relu_t[:],
                    in_=acc[:],
                    func=mybir.ActivationFunctionType.Relu,
                    bias=bias_t[:, :1],
                )
                plin = plin_pool.tile([P, P], f32, space="PSUM", tag="plin")
                nc.tensor.matmul(
                    out=plin[:1, :],
                    lhsT=lin_t[:],
                    rhs=relu_t[:],
                    start=True,
                    stop=True,
                )
                nc.vector.tensor_scalar(
                    out=scores_t[:1, t * P : (t + 1) * P],
                    in0=plin[:1, :],
                    scalar1=linb_t[:1, :1],
                    scalar2=None,
                    op0=mybir.AluOpType.add,
                )
            nc.sync.dma_start(out=scores[:], in_=scores_t[:])

    nc.compile()
    return nc


def _bin_edges(ei, et):
    """Per-graph bin ids and per-edge fields (no padding decisions here)."""
    src = ei[0].astype(np.int64)
    dst = ei[1].astype(np.int64)
    et = et.astype(np.int64)
    cnt = np.bincount(et * N + dst, minlength=R * N).astype(np.float32)
    alpha_e = 1.0 / cnt[et * N + dst]
    t_e = dst >> 7
    h_e = et >> 3
    rl_e = et & 7
    binid = t_e * NH + h_e
    return src, dst, rl_e, alpha_e, binid


def _pack_core_inputs(x, src, dst, rl_e, alpha_e, binid, nch, off,
                      rel_w, root_w, rgcn_b, lin_w, lin_b):
    """Host-side prep for one graph given shared per-bin chunk counts."""
    G = int(off[-1])
    order = np.argsort(binid, kind="stable")
    counts = np.bincount(binid, minlength=NBINS)
    starts = np.zeros(NBINS, np.int64)
    starts[1:] = np.cumsum(counts)[:-1]
    # slot of each (sorted) edge inside the padded chunk layout
    pos = np.arange(E) - starts[binid[order]] + off[binid[order]] * P

    g = np.zeros(G * P, np.int16)  # pad slots gather row 0 (alpha 0)
    so = src[order]
    g[pos] = ((so & 127) * 256 + (so >> 7) * 8 + rl_e[order]).astype(np.int16)
    gidx = np.tile(g.reshape(-1, 16).T, (8, 1)).copy()

    A = np.zeros((G * P, P), np.float32)
    A[pos, (dst[order] & 127)] = alpha_e[order]
    oh = np.ascontiguousarray(
        A.reshape(G, P, P).transpose(1, 0, 2).reshape(P, G * P)
    ).astype(BF16)

    return {
        "xT": np.ascontiguousarray(x.T).astype(BF16),
        "wcat": np.ascontiguousarray(
            rel_w.transpose(1, 0, 2).reshape(C, R * C)
        ).astype(BF16),
        "root": np.ascontiguousarray(root_w).astype(BF16),
        "bias": np.ascontiguousarray(rgcn_b.reshape(C, 1)).astype(np.float32),
        "lin": np.ascontiguousarray(lin_w.reshape(C, 1)).astype(BF16),
        "linb": np.ascontiguousarray(lin_b.reshape(1, 1)).astype(np.float32),
        "gidx": gidx,
        "oh": oh,
    }


def _prep(node_features, edge_index, edge_type, rel_weight, root_weight,
          rgcn_bias, lin_weight, lin_bias):
    node_features = np.asarray(node_features, np.float32)
    edge_index = np.asarray(edge_index)
    edge_type = np.asarray(edge_type)
    rel_weight = np.asarray(rel_weight, np.float32)
    root_weight = np.asarray(root_weight, np.float32)
    rgcn_bias = np.asarray(rgcn_bias, np.float32)
    lin_weight = np.asarray(lin_weight, np.float32)
    lin_bias = np.asarray(lin_bias, np.float32)

    per_core = [
        _bin_edges(edge_index[b], edge_type[b]) for b in range(B)
    ]
    counts = np.stack(
        [np.bincount(pc[4], minlength=NBINS) for pc in per_core]
    )  # [B, NBINS]
    nch = tuple(int(v) for v in np.maximum(
        1, -(-counts.max(axis=0) // P)
    ))  # shared per-bin chunk counts
    off = np.zeros(NBINS + 1, np.int64)
    off[1:] = np.cumsum(nch)

    in_maps = [
        _pack_core_inputs(
            node_features[b], *per_core[b], nch, off,
            rel_weight, root_weight, rgcn_bias, lin_weight, lin_bias,
        )
        for b in range(B)
    ]
    return nch, in_maps


def kernel(node_features, edge_index, edge_type, rel_weight, root_weight,
           rgcn_bias, lin_weight, lin_bias, **_ignored):
    nch, in_maps = _prep(node_features, edge_index, edge_type, rel_weight,
                         root_weight, rgcn_bias, lin_weight, lin_bias)
    if nch not in _prog_cache:
        _prog_cache[nch] = build_program(nch)
    nc = _prog_cache[nch]
    res = run_bass_kernel_spmd(nc, in_maps, core_ids=list(range(B)))
    out = np.stack([res.results[b]["scores"].reshape(N) for b in range(B)])
    return out.astype(np.float32)


def kernel_profiled(node_features, edge_index, edge_type, rel_weight,
                    root_weight, rgcn_bias, lin_weight, lin_bias, **_ignored):
    """Run once with NTFF tracing; returns exec_time_ns (or None)."""
    import tempfile

    nch, in_maps = _prep(node_features, edge_index, edge_type, rel_weight,
                         root_weight, rgcn_bias, lin_weight, lin_bias)
    if nch not in _prog_cache:
        _prog_cache[nch] = build_program(nch)
    nc = _prog_cache[nch]
    tmpdir = tempfile.mkdtemp(prefix="rgcn_prof_")
    res = run_bass_kernel_spmd(
        nc, in_maps, core_ids=list(range(B)), trace=True, tmpdir=tmpdir
    )
    print(f"profile artifacts in {tmpdir}")
    return res.exec_time_ns


# revision 40
# speedup vs baseline: 1.1514x; 1.1514x over previous
"""RGCN graph-scoring kernel for Trainium2 (8 NeuronCores, one graph per core).

Math (per graph):
  out = relu(x @ root + bias + sum_r mean_r @ W_r);  scores = out @ lin + linb
  mean_r[n] = mean of x[src_e] over edges e with dst_e == n, type_e == r.

Device strategy per core (v2 — bf16 + 4 SWDGE queues + host-built one-hots):
  1. Phase 1: xw[src*8 + r_local] = (x @ W_r)[src] in bf16, staged to DRAM
     (two halves r<8 / r>=8 so gather indices fit in int16). PSUM->SBUF
     copies alternate between the Scalar and Vector engines.
  2. Phase 2, per dst tile t: dma_gather edge rows z_e = xw[src_e, type_e]
     (gathers round-robin across 4 SWDGE queues so descriptor generation
     runs on multiple Q7 core pairs concurrently), then accumulate
       acc[c', m] += z_chunk^T @ OH_chunk
     where OH_chunk[e, m] = alpha_e * (dstloc_e == m) is PRECOMPUTED ON THE
     HOST in bf16 and DMA-streamed (alpha_e = 1/cnt(type_e, dst_e) folds the
     mean normalization; padding slots have alpha 0 and index 0).
     acc is seeded by the root-weight matmul; relu+bias on ACT; scores via
     an M=1 matmul with the linear head.

Host side shards graphs across cores, sorts edges by (dst_tile, r_half) into
per-bin chunk counts shared across all 8 cores (max over cores, rounded up to
128) so the SPMD program is identical, and packs index/one-hot arrays.
"""

import sys

for _p in ("/opt/trn_rl_repo", "/root/.axon_site/_ro/trn_rl_repo"):
    if _p not in sys.path:
        sys.path.insert(0, _p)

import numpy as np
import ml_dtypes

import concourse.bacc as bacc
import concourse.mybir as mybir
from concourse.tile import TileContext
from concourse.bass_utils import run_bass_kernel_spmd

P = 128
B, N, C, R, E = 8, 4096, 128, 16, 65536
NT = N // P  # 32 dst node tiles
NH = 2  # r halves (int16 gather index limit: src*8+rl < 32768)
RH = R // NH  # 8 relations per half
NBINS = NT * NH
NQ = 4  # SWDGE queues
Z1B = 12  # z1 pool depth (h=1 gather buffers)

BF16 = ml_dtypes.bfloat16

_prog_cache = {}


def build_program(nch):
    """Build + compile the SPMD Bass program for per-bin chunk counts `nch`
    (tuple of NBINS ints, shared by all cores)."""
    nch = tuple(int(v) for v in nch)
    G = sum(nch)  # total 128-edge chunks
    off = np.zeros(NBINS + 1, np.int64)
    off[1:] = np.cumsum(nch)

    nc = bacc.Bacc("TRN2", num_swdge_queues=NQ)
    f32 = mybir.dt.float32
    bf16 = mybir.dt.bfloat16

    xT = nc.dram_tensor("xT", [P, N], bf16, kind="ExternalInput")
    wcat = nc.dram_tensor("wcat", [P, R * C], bf16, kind="ExternalInput")
    root = nc.dram_tensor("root", [P, C], bf16, kind="ExternalInput")
    bias = nc.dram_tensor("bias", [P, 1], f32, kind="ExternalInput")
    lin = nc.dram_tensor("lin", [P, 1], bf16, kind="ExternalInput")
    linb = nc.dram_tensor("linb", [1, 1], f32, kind="ExternalInput")
    gidx = nc.dram_tensor("gidx", [P, G * 8], mybir.dt.int16, kind="ExternalInput")
    oh = nc.dram_tensor("oh", [P, G * P], bf16, kind="ExternalInput")
    scores = nc.dram_tensor("scores", [1, N], f32, kind="ExternalOutput")

    with TileContext(nc) as tc:
        with (
            tc.tile_pool(name="const", bufs=1) as cpool,
            tc.tile_pool(name="stage", bufs=3) as spool,
            tc.tile_pool(name="z0", bufs=NT) as zpool0,
            tc.tile_pool(name="z1", bufs=Z1B) as zpool1,
            tc.tile_pool(name="oh", bufs=6) as ohpool,
            tc.tile_pool(name="post", bufs=4) as ppool,
            tc.tile_pool(name="pxw", bufs=3, space="PSUM") as pxw_pool,
            tc.tile_pool(name="pacc", bufs=4, space="PSUM") as pacc_pool,
            tc.tile_pool(name="plin", bufs=1, space="PSUM") as plin_pool,
            tc.tile_pool(name="dram", bufs=1, space="DRAM") as dpool,
        ):
            # ---- resident inputs ----
            # xT/wcat gate the first phase-1 matmul: split them across the two
            # HWDGE rings so they land as early as possible.
            xT_t = cpool.tile([P, N], bf16)
            nc.sync.dma_start(out=xT_t[:, : N // 2], in_=xT[:, : N // 2])
            nc.scalar.dma_start(out=xT_t[:, N // 2 :], in_=xT[:, N // 2 :])
            wcat_t = cpool.tile([P, R * C], bf16)
            nc.sync.dma_start(out=wcat_t[:], in_=wcat[:])
            root_t = cpool.tile([P, C], bf16)
            nc.sync.dma_start(out=root_t[:], in_=root[:])
            bias_t = cpool.tile([P, 1], f32)
            nc.sync.dma_start(out=bias_t[:], in_=bias[:])
            lin_t = cpool.tile([P, 1], bf16)
            nc.sync.dma_start(out=lin_t[:], in_=lin[:])
            linb_t = cpool.tile([1, 1], f32)
            nc.sync.dma_start(out=linb_t[:], in_=linb[:])
            # idx feeds the gathers (not needed before ~40us) — keep it off the
            # Sync ring's critical phase-1a stretch.
            idx_t = cpool.tile([P, G * 8], mybir.dt.int16)
            nc.scalar.dma_start(out=idx_t[:], in_=gidx[:])

            # DRAM scratch: per-half transformed features, row = src*8 + r_local
            xw = [
                dpool.tile([N * RH, C], bf16, name=f"xw{h}", tag=f"xw{h}")
                for h in range(NH)
            ]

            # Pre-issue the first OH loads on the ACT HWDGE ring (Sync's ring
            # carries the inputs + xw[0] writes) — they have no dependencies
            # and phase 2's first accumulations need them early.
            oh_tiles = [None] * NT
            OH_PRE = 6
            for t in range(OH_PRE):
                b0 = t * NH
                tch = nch[b0] + nch[b0 + 1]
                oh_tiles[t] = ohpool.tile([P, tch * P], bf16, name=f"oh{t}", tag="oh")
                nc.scalar.dma_start(
                    out=oh_tiles[t][:],
                    in_=oh[:, off[b0] * P : (off[b0] + tch) * P],
                )

            # ---- phase 1: xw = x @ W_r (bf16), staged out to DRAM ----
            # h-major so all of xw[0] lands first and the h=0 gathers can
            # start while the h=1 half is still being computed. xw writes are
            # batched 4 src tiles per DMA to keep the Sync engine's issue rate
            # off the critical path.
            SG = 4  # src tiles per staged write
            for h in range(NH):
                for sg in range(NT // SG):
                    stg = spool.tile([P, SG, RH * C], bf16, tag="stage")
                    for si in range(SG):
                        st = sg * SG + si
                        for g in range(2):
                            pxw = pxw_pool.tile([P, 512], f32, space="PSUM", tag="pxw")
                            nc.tensor.matmul(
                                out=pxw[:],
                                lhsT=xT_t[:, st * P : (st + 1) * P],
                                rhs=wcat_t[:, h * RH * C + g * 512 : h * RH * C + (g + 1) * 512],
                                start=True,
                                stop=True,
                            )
                            # alternate ACT/DVE on the PSUM->SBUF cast so
                            # neither engine gates the PE
                            if g == 0:
                                nc.scalar.activation(
                                    out=stg[:, si, :512],
                                    in_=pxw[:],
                                    func=mybir.ActivationFunctionType.Copy,
                                )
                            else:
                                nc.vector.tensor_scalar(
                                    out=stg[:, si, 512:],
                                    in0=pxw[:],
                                    scalar1=0.0,
                                    scalar2=None,
                                    op0=mybir.AluOpType.add,
                                )
                    # xw row index is (src%128)*256 + (src//128)*8 + rl so each
                    # staged write is one contiguous 8KB run per partition
                    # (full-bandwidth DMA shape); the host computes gather
                    # indices with the same mapping.
                    dst_view = xw[h][:].rearrange(
                        "(p ng s rl) c -> ng p (s rl c)", p=P, ng=NT // SG, s=SG, rl=RH
                    )[sg]
                    # split write traffic across the two HWDGE rings: xw[0] on
                    # Sync (with the inputs), xw[1] on ACT (after OH preloads)
                    eng = nc.sync if h == 0 else nc.scalar
                    eng.dma_start(
                        out=dst_view,
                        in_=stg[:].rearrange("p s (rl c) -> p (s rl c)", rl=RH),
                    )

            # ---- phase 2: gather + aggregate per dst tile ----
            # All h=0 gathers are issued first: they only depend on xw[0], so
            # Q7 descriptor generation overlaps with phase 1's h=1 half.
            z_h0 = []
            for t in range(NT):
                b = t * NH
                cap = nch[b] * P
                z = zpool0.tile([P, nch[b], C], bf16, name=f"z0_{t}", tag="z0")
                nc.gpsimd.dma_gather(
                    z[:],
                    xw[0][:],
                    idx_t[:, off[b] * 8 : off[b] * 8 + cap // 16],
                    cap,
                    cap,
                    C,
                    single_packet=False,
                    queue_num=t % NQ,
                )
                z_h0.append(z)

            scores_t = cpool.tile([1, N], f32)
            for t in range(NT):
                b0, b1 = t * NH, t * NH + 1
                tch = nch[b0] + nch[b1]  # chunks for this tile
                if oh_tiles[t] is None:
                    oh_tiles[t] = ohpool.tile([P, tch * P], bf16, name=f"oh{t}", tag="oh")
                    nc.sync.dma_start(
                        out=oh_tiles[t][:],
                        in_=oh[:, off[b0] * P : (off[b0] + tch) * P],
                    )
                oh_t = oh_tiles[t]
                cap1 = nch[b1] * P
                z1 = zpool1.tile([P, nch[b1], C], bf16, name=f"z1_{t}", tag="z1")
                nc.gpsimd.dma_gather(
                    z1[:],
                    xw[1][:],
                    idx_t[:, off[b1] * 8 : off[b1] * 8 + cap1 // 16],
                    cap1,
                    cap1,
                    C,
                    single_packet=False,
                    queue_num=t % NQ,
                )
                acc = pacc_pool.tile([P, P], f32, space="PSUM", tag="acc")
                # root term seeds the accumulator (start=True clears the bank)
                nc.tensor.matmul(
                    out=acc[:],
                    lhsT=root_t[:],
                    rhs=xT_t[:, t * P : (t + 1) * P],
                    start=True,
                    stop=False,
                )
                for h, zt in ((0, z_h0[t]), (1, z1)):
                    b = t * NH + h
                    for j in range(nch[b]):
                        g_loc = (off[b] - off[b0]) + j
                        nc.tensor.matmul(
                            out=acc[:],
                            lhsT=zt[:, j, :],
                            rhs=oh_t[:, g_loc * P : (g_loc + 1) * P],
                            start=False,
                            stop=(h == NH - 1 and j == nch[b] - 1),
                        )
                # relu(acc + bias) -> SBUF bf16
                relu_t = ppool.tile([P, P], bf16, tag="relu")
                nc.scalar.activation(
                    out=relu_t[:],
                    in_=acc[:],
                    func=mybir.ActivationFunctionType.Relu,
                    bias=bias_t[:, :1],
                )
                plin = plin_pool.tile([P, P], f32, space="PSUM", tag="plin")
                nc.tensor.matmul(
                    out=plin[:1, :],
                    lhsT=lin_t[:],
                    rhs=relu_t[:],
                    start=True,
                    stop=True,
                )
                nc.vector.tensor_scalar(
                    out=scores_t[:1, t * P : (t + 1) * P],
                    in0=plin[:1, :],
                    scalar1=linb_t[:1, :1],
                    scalar2=None,
                    op0=mybir.AluOpType.add,
                )
            nc.sync.dma_start(out=scores[:], in_=scores_t[:])

    nc.compile()
    return nc


def _bin_edges(ei, et):
    """Per-graph bin ids and per-edge fields (no padding decisions here)."""
    src = ei[0].astype(np.int64)
    dst = ei[1].astype(np.int64)
    et = et.astype(np.int64)
    cnt = np.bincount(et * N + dst, minlength=R * N).astype(np.float32)
    alpha_e = 1.0 / cnt[et * N + dst]
    t_e = dst >> 7
    h_e = et >> 3
    rl_e = et & 7
    binid = t_e * NH + h_e
    return src, dst, rl_e, alpha_e, binid


def _pack_core_inputs(x, src, dst, rl_e, alpha_e, binid, nch, off,
                      rel_w, root_w, rgcn_b, lin_w, lin_b):
    """Host-side prep for one graph given shared per-bin chunk counts."""
    G = int(off[-1])
    order = np.argsort(binid, kind="stable")
    counts = np.bincount(binid, minlength=NBINS)
    starts = np.zeros(NBINS, np.int64)
    starts[1:] = np.cumsum(counts)[:-1]
    # slot of each (sorted) edge inside the padded chunk layout
    pos = np.arange(E) - starts[binid[order]] + off[binid[order]] * P

    g = np.zeros(G * P, np.int16)  # pad slots gather row 0 (alpha 0)
    so = src[order]
    g[pos] = ((so & 127) * 256 + (so >> 7) * 8 + rl_e[order]).astype(np.int16)
    gidx = np.tile(g.reshape(-1, 16).T, (8, 1)).copy()

    A = np.zeros((G * P, P), np.float32)
    A[pos, (dst[order] & 127)] = alpha_e[order]
    oh = np.ascontiguousarray(
        A.reshape(G, P, P).transpose(1, 0, 2).reshape(P, G * P)
    ).astype(BF16)

    return {
        "xT": np.ascontiguousarray(x.T).astype(BF16),
        "wcat": np.ascontiguousarray(
            rel_w.transpose(1, 0, 2).reshape(C, R * C)
        ).astype(BF16),
        "root": np.ascontiguousarray(root_w).astype(BF16),
        "bias": np.ascontiguousarray(rgcn_b.reshape(C, 1)).astype(np.float32),
        "lin": np.ascontiguousarray(lin_w.reshape(C, 1)).astype(BF16),
        "linb": np.ascontiguousarray(lin_b.reshape(1, 1)).astype(np.float32),
        "gidx": gidx,
        "oh": oh,
    }


def _prep(node_features, edge_index, edge_type, rel_weight, root_weight,
          rgcn_bias, lin_weight, lin_bias):
    node_features = np.asarray(node_features, np.float32)
    edge_index = np.asarray(edge_index)
    edge_type = np.asarray(edge_type)
    rel_weight = np.asarray(rel_weight, np.float32)
    root_weight = np.asarray(root_weight, np.float32)
    rgcn_bias = np.asarray(rgcn_bias, np.float32)
    lin_weight = np.asarray(lin_weight, np.float32)
    lin_bias = np.asarray(lin_bias, np.float32)

    per_core = [
        _bin_edges(edge_index[b], edge_type[b]) for b in range(B)
    ]
    counts = np.stack(
        [np.bincount(pc[4], minlength=NBINS) for pc in per_core]
    )  # [B, NBINS]
    nch = tuple(int(v) for v in np.maximum(
        1, -(-counts.max(axis=0) // P)
    ))  # shared per-bin chunk counts
    off = np.zeros(NBINS + 1, np.int64)
    off[1:] = np.cumsum(nch)

    in_maps = [
        _pack_core_inputs(
            node_features[b], *per_core[b], nch, off,
            rel_weight, root_weight, rgcn_bias, lin_weight, lin_bias,
        )
        for b in range(B)
    ]
    return nch, in_maps


def kernel(node_features, edge_index, edge_type, rel_weight, root_weight,
           rgcn_bias, lin_weight, lin_bias, **_ignored):
    nch, in_maps = _prep(node_features, edge_index, edge_type, rel_weight,
                         root_weight, rgcn_bias, lin_weight, lin_bias)
    if nch not in _prog_cache:
        _prog_cache[nch] = build_program(nch)
    nc = _prog_cache[nch]
    res = run_bass_kernel_spmd(nc, in_maps, core_ids=list(range(B)))
    out = np.stack([res.results[b]["scores"].reshape(N) for b in range(B)])
    return out.astype(np.float32)


def kernel_profiled(node_features, edge_index, edge_type, rel_weight,
                    root_weight, rgcn_bias, lin_weight, lin_bias, **_ignored):
    """Run once with NTFF tracing; returns exec_time_ns (or None)."""
    import tempfile

    nch, in_maps = _prep(node_features, edge_index, edge_type, rel_weight,
                         root_weight, rgcn_bias, lin_weight, lin_bias)
    if nch not in _prog_cache:
        _prog_cache[nch] = build_program(nch)
    nc = _prog_cache[nch]
    tmpdir = tempfile.mkdtemp(prefix="rgcn_prof_")
    res = run_bass_kernel_spmd(
        nc, in_maps, core_ids=list(range(B)), trace=True, tmpdir=tmpdir
    )
    print(f"profile artifacts in {tmpdir}")
    return res.exec_time_ns
